# revision 23
# baseline (speedup 1.0000x reference)
"""Grouped-Query Attention (B=2, S=2048, D=2048, 16 Q heads / 4 KV heads,
hd=128, RoPE, causal) on 8 trn2 NeuronCores.

Sharding: mesh = 2 (batch) x 4 (KV-head groups).  Core c = b*4 + g gets
batch b and KV head g together with its 4 query heads (tensor parallel on
the head dim: q/k/v projection output dim and o-proj input dim).  Each core
produces a partial y[b] (o-proj over its 512 input dims); host sums the 4
partials per batch.

v3: all matmul operands bf16 (1 cycle/row + fast weight load; absmax rel
err ~4e-3 vs the 2e-2 gate).  Phase 1 serializes the contraction per
projection output (2 PSUM banks live) and lags each output's RoPE /
v-transpose epilogue one 16-matmul block behind, so the PE never waits on
ACT/DVE.  Phase 2 lags the softmax-denominator and attention@V matmuls of
head h-1 into head h's scores block (3 interleaved PE streams), and the
1/den -> broadcast -> normalize chain a further block, so the PE never
waits on the exp chain.  The reciprocal runs as ACT ln->exp(-x) on the
small early sq blocks (DVE is the scarce engine there) and as DVE
reciprocal on the big ones (ACT is scarce there).  DMAs are chunked and
priority-ordered so the first matmul issues ~12us in (8.6us of that is
fixed NEFF preamble).
"""

import os

import numpy as np

S = 2048
D = 2048
HD = 128
NQH = 16
NKVH = 4
GROUPS = NQH // NKVH  # 4 q heads per kv head
O = GROUPS * HD  # 512 per-core q/o slice
NB = 2
NCORES = 8
SCALE = 1.0 / float(np.sqrt(np.float32(HD)))
NEG = -1.0e30

SBLK = 512  # seq block for projections / sq block in attention
NKB = S // HD  # 16 128-blocks along seq
NSB = S // SBLK  # 4 512-blocks along seq
NDB = D // HD  # 16 d blocks

LAST_EXEC_NS = None
LAST_TRACE = None

_CACHE = {}


def _rope_tables():
    k = np.arange(0, HD, 2)[: HD // 2].astype(np.float32)
    inv_freq = (1.0 / 10000.0 ** (k / HD)).astype(np.float32)
    positions = np.arange(S, dtype=np.float32)
    ang = positions[:, None] * inv_freq[None, :]  # [S, 64]
    ang = np.concatenate([ang, ang], axis=-1)  # [S, 128]
    cosT = np.cos(ang).astype(np.float32).T  # [128, S]
    sinT = np.sin(ang).astype(np.float32).T
    return np.ascontiguousarray(cosT), np.ascontiguousarray(sinT)


def _mask_table():
    # maskT[i, j*512 + s] = 0 if (j*128 + i) <= s else NEG
    m = np.empty((HD, 4 * SBLK), dtype=np.float32)
    i = np.arange(HD)[:, None]
    s = np.arange(SBLK)[None, :]
    for j in range(4):
        m[:, j * SBLK : (j + 1) * SBLK] = np.where(j * HD + i <= s, 0.0, NEG)
    return m


def _shift_table():
    # rot = P @ q  with rot[i] = -q[i+64] (i<64), q[i-64] (i>=64); ship P.T
    P = np.zeros((HD, HD), dtype=np.float32)
    h = HD // 2
    P[np.arange(h), np.arange(h) + h] = -1.0
    P[np.arange(h) + h, np.arange(h)] = 1.0
    return np.ascontiguousarray(P.T)


def _build_program():
    import concourse.bass as bass
    import concourse.mybir as mybir
    from concourse.tile import TileContext

    f32 = mybir.dt.float32
    f32r = mybir.dt.float32r
    bf16 = mybir.dt.bfloat16
    EXP = mybir.ActivationFunctionType.Exp
    LN = mybir.ActivationFunctionType.Ln

    nc = bass.Bass()

    xP = nc.declare_dram_parameter("xP", [128, NSB * NDB * SBLK], bf16, isOutput=False)
    wqP = nc.declare_dram_parameter(
        "wqP", [128, GROUPS * NDB * HD], bf16, isOutput=False
    )
    wkP = nc.declare_dram_parameter("wkP", [128, NDB * HD], bf16, isOutput=False)
    wvP = nc.declare_dram_parameter("wvP", [128, NDB * HD], bf16, isOutput=False)
    woP = nc.declare_dram_parameter("woP", [128, GROUPS * D], bf16, isOutput=False)
    cosT = nc.declare_dram_parameter("cosT", [HD, S], bf16, isOutput=False)
    sinT = nc.declare_dram_parameter("sinT", [HD, S], bf16, isOutput=False)
    maskT = nc.declare_dram_parameter("maskT", [HD, 4 * SBLK], bf16, isOutput=False)
    shiftPT = nc.declare_dram_parameter("shiftPT", [HD, HD], bf16, isOutput=False)
    ident = nc.declare_dram_parameter("ident", [HD, HD], bf16, isOutput=False)
    onescol = nc.declare_dram_parameter("onescol", [HD, 1], bf16, isOutput=False)
    onesrow = nc.declare_dram_parameter("onesrow", [1, HD], f32r, isOutput=False)
    onesrowb = nc.declare_dram_parameter("onesrowb", [1, HD], bf16, isOutput=False)
    y = nc.declare_dram_parameter("y", [S, D], f32, isOutput=True)

    with TileContext(nc) as tc:
        with tc.tile_pool(name="persist", bufs=1) as pp:
            wq_sb = pp.tile([128, GROUPS * NDB * HD], bf16, name="wq_sb")
            wk_sb = pp.tile([128, NDB * HD], bf16, name="wk_sb")
            wv_sb = pp.tile([128, NDB * HD], bf16, name="wv_sb")
            wo_sb = pp.tile([128, GROUPS * D], bf16, name="wo_sb")
            cos_sb = pp.tile([128, S], bf16, name="cos_sb")
            sin_sb = pp.tile([128, S], bf16, name="sin_sb")
            mask_sb = pp.tile([128, 4 * SBLK], bf16, name="mask_sb")
            shift_sb = pp.tile([128, HD], bf16, name="shift_sb")
            id_sb = pp.tile([128, HD], bf16, name="id_sb")
            ones_sb = pp.tile([128, 1], bf16, name="ones_sb")
            oner_sb = pp.tile([1, HD], f32r, name="oner_sb")
            onerb_sb = pp.tile([1, HD], bf16, name="onerb_sb")
            q_sb = pp.tile([128, GROUPS * S], bf16, name="q_sb")  # per head [hd, S]
            k_sb = pp.tile([128, S], bf16, name="k_sb")
            v_sb = pp.tile([128, NKB * HD], bf16, name="v_sb")  # [s_blk][128s, hd]

            # ---- priority-ordered input DMAs.  First compute block is the
            # v projection of sb0, so wv + the first x chunks come first.
            nc.sync.dma_start(out=wv_sb[:], in_=wvP[:])

            with (
                tc.tile_pool(name="xts", bufs=2) as xpool,
                tc.tile_pool(name="p1acc", bufs=3, space="PSUM") as accp,
                tc.tile_pool(name="p1vacc", bufs=2, space="PSUM") as vaccp,
                tc.tile_pool(name="p1rot", bufs=2, space="PSUM") as rotp,
                tc.tile_pool(name="p1vt", bufs=1, space="PSUM") as vtp,
                tc.tile_pool(name="raws", bufs=3) as rawpool,
                tc.tile_pool(name="tmps", bufs=3) as tmppool,
            ):
                xsb0 = xpool.tile([128, NDB * SBLK], bf16, name="xt0", tag="xt")
                for c in range(8):
                    nc.sync.dma_start(
                        out=xsb0[:, c * 1024 : (c + 1) * 1024],
                        in_=xP[:, c * 1024 : (c + 1) * 1024],
                    )
                    if c == 1:
                        nc.sync.dma_start(
                            out=wq_sb[:, 0:2048], in_=wqP[:, 0:2048]
                        )  # q head 0
                    if c == 3:
                        nc.sync.dma_start(out=id_sb[:], in_=ident[:])
                        nc.sync.dma_start(out=cos_sb[:, 0:SBLK], in_=cosT[:, 0:SBLK])
                    if c == 5:
                        nc.sync.dma_start(out=sin_sb[:, 0:SBLK], in_=sinT[:, 0:SBLK])
                        nc.sync.dma_start(out=shift_sb[:], in_=shiftPT[:])
                for ob in range(1, GROUPS):
                    nc.sync.dma_start(
                        out=wq_sb[:, ob * 2048 : (ob + 1) * 2048],
                        in_=wqP[:, ob * 2048 : (ob + 1) * 2048],
                    )
                nc.sync.dma_start(out=wk_sb[:], in_=wkP[:])
                nc.sync.dma_start(out=ones_sb[:], in_=onescol[:])
                nc.sync.dma_start(out=oner_sb[:], in_=onesrow[:])
                nc.sync.dma_start(out=onerb_sb[:], in_=onesrowb[:])

                def rope(ps, raw_name, dst, seq_sl):
                    # dst = raw*cos + (P@raw)*sin, all bf16 except rot PSUM f32
                    raw = rawpool.tile([128, SBLK], bf16, name=raw_name, tag="raw")
                    nc.scalar.copy(raw[:], ps[:])
                    rot = rotp.tile([128, SBLK], f32, name="rot_" + raw_name, tag="rot")
                    nc.tensor.matmul(rot[:], shift_sb[:], raw[:], start=True, stop=True)
                    d = dst[0][:, dst[1]]
                    nc.vector.tensor_mul(d, raw[:], cos_sb[:, seq_sl])
                    t2 = tmppool.tile([128, SBLK], bf16, name="t2_" + raw_name, tag="t2")
                    nc.vector.tensor_mul(t2[:], rot[:], sin_sb[:, seq_sl])
                    nc.vector.tensor_add(d, d, t2[:])

                def vfinish(pv, sb):
                    # stage v to SBUF bf16, PE-transpose to [seq, hd] blocks
                    vst = rawpool.tile([128, SBLK], bf16, name=f"vst{sb}", tag="raw")
                    nc.scalar.copy(vst[:], pv[:])
                    vt = vtp.tile([128, SBLK], bf16, name=f"vt{sb}", tag="vt")
                    for sub in range(SBLK // HD):
                        nc.tensor.transpose(
                            vt[:, sub * HD : (sub + 1) * HD],
                            vst[:, sub * HD : (sub + 1) * HD],
                            id_sb[:],
                        )
                    nc.scalar.copy(v_sb[:, sb * SBLK : (sb + 1) * SBLK], vt[:])

                # block order per sb: v, q0..q3, k (sb0 fuses v+q0 so early
                # compute matches x-DMA delivery; sb3 puts v last so the final
                # phase-1 epilogue is the v transpose, which phase 2 does not
                # read until several blocks in).  Each block's epilogue (RoPE
                # or v transpose) is emitted after a LATER block's matmuls via
                # a FIFO so the PE never waits on ACT/DVE.
                taskq = []

                def flush_one():
                    if taskq:
                        fn, args = taskq.pop(0)
                        fn(*args)

                for sb in range(NSB):
                    sl = slice(sb * SBLK, (sb + 1) * SBLK)
                    if sb > 0:
                        xsb = xpool.tile([128, NDB * SBLK], bf16, name=f"xt{sb}", tag="xt")
                        for c in range(4):
                            nc.sync.dma_start(
                                out=xsb[:, c * 2048 : (c + 1) * 2048],
                                in_=xP[
                                    :, sb * 8192 + c * 2048 : sb * 8192 + (c + 1) * 2048
                                ],
                            )
                        nc.sync.dma_start(out=cos_sb[:, sl], in_=cosT[:, sl])
                        nc.sync.dma_start(out=sin_sb[:, sl], in_=sinT[:, sl])
                        if sb == 1:
                            nc.sync.dma_start(out=mask_sb[:], in_=maskT[:])
                        if sb == 2:
                            nc.sync.dma_start(out=wo_sb[:], in_=woP[:])
                    else:
                        xsb = xsb0

                    def vblock(sb=sb, xsb=xsb):
                        pv = vaccp.tile([128, SBLK], f32, name=f"vacc{sb}", tag="vacc")
                        for db in range(NDB):
                            nc.tensor.matmul(
                                pv[:],
                                wv_sb[:, db * HD : (db + 1) * HD],
                                xsb[:, db * SBLK : (db + 1) * SBLK],
                                start=(db == 0),
                                stop=(db == NDB - 1),
                            )
                        return pv

                    oi_start = 0
                    if sb == 0:
                        # fused v + q0 contraction over the incoming x chunks
                        pv = vaccp.tile([128, SBLK], f32, name="vacc0", tag="vacc")
                        ps0 = accp.tile([128, SBLK], f32, name="acc0_0", tag="acc")
                        for db in range(NDB):
                            nc.tensor.matmul(
                                pv[:],
                                wv_sb[:, db * HD : (db + 1) * HD],
                                xsb[:, db * SBLK : (db + 1) * SBLK],
                                start=(db == 0),
                                stop=(db == NDB - 1),
                            )
                            nc.tensor.matmul(
                                ps0[:],
                                wq_sb[:, db * HD : (db + 1) * HD],
                                xsb[:, db * SBLK : (db + 1) * SBLK],
                                start=(db == 0),
                                stop=(db == NDB - 1),
                            )
                        taskq.append((vfinish, (pv, sb)))
                        taskq.append((rope, (ps0, "r0_0", (q_sb, slice(0, SBLK)), sl)))
                        oi_start = 1
                    elif sb < 3:
                        pv = vblock()
                        flush_one()
                        taskq.append((vfinish, (pv, sb)))

                    # q heads then k: serialized 16-step contraction per output
                    for oi in range(oi_start, 5):
                        ps = accp.tile([128, SBLK], f32, name=f"acc{sb}_{oi}", tag="acc")
                        for db in range(NDB):
                            if oi < 4:
                                w = wq_sb[
                                    :, oi * 2048 + db * HD : oi * 2048 + (db + 1) * HD
                                ]
                            else:
                                w = wk_sb[:, db * HD : (db + 1) * HD]
                            nc.tensor.matmul(
                                ps[:],
                                w,
                                xsb[:, db * SBLK : (db + 1) * SBLK],
                                start=(db == 0),
                                stop=(db == NDB - 1),
                            )
                        flush_one()
                        dst = (
                            (q_sb, slice(oi * S + sb * SBLK, oi * S + (sb + 1) * SBLK))
                            if oi < 4
                            else (k_sb, sl)
                        )
                        taskq.append((rope, (ps, f"r{sb}_{oi}", dst, sl)))
                    if sb == 3:
                        pv = vblock()
                        flush_one()
                        taskq.append((vfinish, (pv, sb)))
                while taskq:
                    flush_one()

            # ---------------- Phase 2: attention + o-proj
            with (
                tc.tile_pool(name="p2sc", bufs=3, space="PSUM") as scp,
                tc.tile_pool(name="p2av", bufs=2, space="PSUM") as avp,
                tc.tile_pool(name="p2den", bufs=2, space="PSUM") as denp,
                tc.tile_pool(name="p2bc", bufs=1, space="PSUM") as bcp,
                tc.tile_pool(name="exps", bufs=36) as epool,
                tc.tile_pool(name="denrs", bufs=4) as drpool,
                tc.tile_pool(name="bcsb", bufs=2) as bcsbpool,
                tc.tile_pool(name="aos", bufs=2) as aopool,
                tc.tile_pool(name="ysb", bufs=3) as ypool_sb,
            ):
                ao_tiles = {}

                def emit_recip(sq, h, den, use_act=None):
                    # 1/den: ACT ln->exp(-x) on small sq (DVE-scarce blocks),
                    # DVE reciprocal on big sq (ACT-scarce blocks)
                    if use_act is None:
                        use_act = True
                    if use_act:
                        lden = bcsbpool.tile(
                            [1, SBLK], f32, name=f"ld{sq}_{h}", tag="ld"
                        )
                        nc.scalar.activation(lden[:], den[:], LN)
                        denr = drpool.tile(
                            [1, SBLK], bf16, name=f"denr{sq}_{h}", tag="denr"
                        )
                        nc.scalar.activation(denr[:], lden[:], EXP, scale=-1.0)
                        return (denr, onerb_sb)
                    denr = drpool.tile([1, SBLK], f32r, name=f"denr{sq}_{h}", tag="denr")
                    with nc.allow_low_precision(reason="f32r softmax denom"):
                        nc.vector.reciprocal(denr[:], den[:])
                    return (denr, oner_sb)

                def finish_norm(rec):
                    # bc = broadcast(1/den) via ones-column matmul; ao = av * bc
                    psq, ph, av, (denr, oner) = rec
                    bc = bcp.tile([128, SBLK], f32, name=f"bc{psq}_{ph}", tag="bc")
                    nc.tensor.matmul(bc[:], oner[:], denr[:], start=True, stop=True)
                    bcs = bcsbpool.tile([128, SBLK], f32, name=f"bcs{psq}_{ph}", tag="bcs")
                    nc.scalar.copy(bcs[:], bc[:])
                    nc.vector.tensor_mul(
                        ao_tiles[psq][:, ph * SBLK : (ph + 1) * SBLK], av[:], bcs[:]
                    )

                def emit_oproj(sq):
                    ao = ao_tiles[sq]
                    for sub in range(SBLK // HD):
                        for dc in range(D // SBLK):
                            yt = scp.tile(
                                [128, SBLK], f32, name=f"y{sq}_{sub}_{dc}", tag="sc"
                            )
                            for ob in range(GROUPS):
                                nc.tensor.matmul(
                                    yt[:],
                                    ao[
                                        :, ob * SBLK + sub * HD : ob * SBLK + (sub + 1) * HD
                                    ],
                                    wo_sb[:, ob * D + dc * SBLK : ob * D + (dc + 1) * SBLK],
                                    start=(ob == 0),
                                    stop=(ob == GROUPS - 1),
                                )
                            ysb = ypool_sb.tile(
                                [128, SBLK], f32, name=f"ysb{sq}_{sub}_{dc}", tag="ysb"
                            )
                            if (sub + dc) % 2 == 0:
                                nc.scalar.copy(ysb[:], yt[:])
                            else:
                                nc.vector.tensor_copy(ysb[:], yt[:])
                            nc.sync.dma_start(
                                out=y[
                                    sq * SBLK + sub * HD : sq * SBLK + (sub + 1) * HD,
                                    dc * SBLK : (dc + 1) * SBLK,
                                ],
                                in_=ysb[:],
                            )

                lagA = None  # (sq, h, es): den+av matmuls run in current block
                lagN = None  # (sq, h, av, rec): bc/bcs/normalize run in current block
                for sq in range(NSB):
                    nsk = 4 * sq + 4
                    ao_tiles[sq] = aopool.tile(
                        [128, GROUPS * SBLK], bf16, name=f"ao{sq}", tag="ao"
                    )
                    for h in range(GROUPS):
                        es = []
                        if lagA is not None:
                            asq, ah, aes = lagA
                            psk = len(aes)
                            den = denp.tile([1, SBLK], f32, name=f"den{asq}_{ah}", tag="den")
                            av = avp.tile([128, SBLK], f32, name=f"av{asq}_{ah}", tag="av")
                        else:
                            psk = 0
                        qcols = q_sb[:, h * S + sq * SBLK : h * S + (sq + 1) * SBLK]
                        # columns below j*128 of diagonal tile j are fully
                        # masked (exp -> 0): trim scores/mask/exp/den/av to the
                        # valid column subrange.  es[i] = (tile, col_offset).
                        def co_of(sq_, i_):
                            return max(0, (i_ - 4 * sq_) * HD)

                        for i in range(max(nsk, psk)):
                            if i < nsk:
                                co = co_of(sq, i)
                                sc = scp.tile(
                                    [128, SBLK], f32, name=f"sc{sq}_{h}_{i}", tag="sc"
                                )
                                nc.tensor.matmul(
                                    sc[:, co:SBLK],
                                    k_sb[:, i * HD : (i + 1) * HD],
                                    qcols[:, co:SBLK],
                                    start=True,
                                    stop=True,
                                )
                                if i >= 4 * sq:
                                    j = i - 4 * sq
                                    nc.vector.tensor_add(
                                        sc[:, co:SBLK],
                                        sc[:, co:SBLK],
                                        mask_sb[:, j * SBLK + co : (j + 1) * SBLK],
                                    )
                            if i < psk:
                                pco = co_of(asq, i)
                                nc.tensor.matmul(
                                    den[:, pco:SBLK],
                                    ones_sb[:],
                                    aes[i][0][:, pco:SBLK],
                                    start=(i == 0),
                                    stop=(i == psk - 1),
                                )
                                nc.tensor.matmul(
                                    av[:, pco:SBLK],
                                    v_sb[:, i * HD : (i + 1) * HD],
                                    aes[i][0][:, pco:SBLK],
                                    start=(i == 0),
                                    stop=(i == psk - 1),
                                )
                            if i < nsk:
                                e = epool.tile(
                                    [128, SBLK], bf16, name=f"e{sq}_{h}_{i}", tag="e"
                                )
                                nc.scalar.activation(
                                    e[:, co:SBLK], sc[:, co:SBLK], EXP, scale=SCALE
                                )
                                es.append((e, co))
                        if lagA is not None:
                            rec = emit_recip(asq, ah, den)
                            if lagN is not None:
                                finish_norm(lagN)
                            lagN = (asq, ah, av, rec)
                        lagA = (sq, h, es)
                        if h == 1 and sq > 0:
                            emit_oproj(sq - 1)
                # drain: den+av for (3,3), last normalizations, final o-proj
                asq, ah, aes = lagA
                psk = len(aes)
                den = denp.tile([1, SBLK], f32, name="den3_3", tag="den")
                av = avp.tile([128, SBLK], f32, name="av3_3", tag="av")
                for i in range(psk):
                    pco = max(0, (i - 4 * asq) * HD)
                    nc.tensor.matmul(
                        den[:, pco:SBLK],
                        ones_sb[:],
                        aes[i][0][:, pco:SBLK],
                        start=(i == 0),
                        stop=(i == psk - 1),
                    )
                rec = emit_recip(asq, ah, den, use_act=True)
                for i in range(psk):
                    pco = max(0, (i - 4 * asq) * HD)
                    nc.tensor.matmul(
                        av[:, pco:SBLK],
                        v_sb[:, i * HD : (i + 1) * HD],
                        aes[i][0][:, pco:SBLK],
                        start=(i == 0),
                        stop=(i == psk - 1),
                    )
                finish_norm(lagN)
                finish_norm((asq, ah, av, rec))
                emit_oproj(NSB - 1)

    _split_matmul_waits(nc, mybir)
    return nc


def _split_matmul_waits(nc, mybir):
    """TRN2 instructions can carry only one HW sync-wait command; Tile
    sometimes attaches several.  Move the extras onto nofuse nops on the
    same engine inserted just before the instruction."""
    for f in nc.m.functions:
        for bb in f.blocks:
            insts = bb.instructions
            fixes = []
            for idx, inst in enumerate(insts):
                si = inst.sync_info
                if si is None or len(si.on_wait) <= 1:
                    continue
                fixes.append((idx, inst, list(si.on_wait), list(si.on_update)))
            for idx, inst, waits, updates in reversed(fixes):
                inst.sync_info = mybir.SyncInfo(on_wait=[waits[-1]], on_update=updates)
                for w in reversed(waits[:-1]):
                    nop = mybir.InstNoOp(
                        name=nc.get_next_instruction_name(), ins=[], outs=[]
                    )
                    nop.engine = inst.engine
                    nop.bass_nofuse = True
                    nop.sync_info = mybir.SyncInfo(on_wait=[w], on_update=[])
                    insts.insert(idx, nop)


def _bf16(a):
    import ml_dtypes

    return np.ascontiguousarray(a).astype(ml_dtypes.bfloat16)


def _per_core_inputs(x, Wq, Wk, Wv, Wo):
    cosT, sinT = _rope_tables()
    maskT = _mask_table()
    shiftPT = _shift_table()
    identity = np.eye(HD, dtype=np.float32)
    onescol = np.ones((HD, 1), dtype=np.float32)
    onesrow = np.ones((1, HD), dtype=np.float32)
    cosT = _bf16(cosT)
    sinT = _bf16(sinT)
    maskT = _bf16(maskT)
    shiftPT = _bf16(shiftPT)
    identity = _bf16(identity)
    onescol = _bf16(onescol)
    in_maps = []
    for b in range(NB):
        xT = x[b].T.astype(np.float32, copy=False)  # [D, S]
        xPb = _bf16(
            xT.reshape(NDB, 128, NSB, SBLK).transpose(1, 2, 0, 3).reshape(128, -1)
        )
        for g in range(NKVH):
            wqT = Wq[g * O : (g + 1) * O, :].T  # [D, O]
            wkT = Wk[g * HD : (g + 1) * HD, :].T  # [D, HD]
            wvT = Wv[g * HD : (g + 1) * HD, :].T
            woT = Wo[:, g * O : (g + 1) * O].T  # [O, D]
            in_maps.append(
                {
                    "xP": xPb,
                    "wqP": _bf16(
                        wqT.reshape(NDB, 128, GROUPS, HD)
                        .transpose(1, 2, 0, 3)
                        .reshape(128, -1)
                    ),
                    "wkP": _bf16(
                        wkT.reshape(NDB, 128, HD).transpose(1, 0, 2).reshape(128, -1)
                    ),
                    "wvP": _bf16(
                        wvT.reshape(NDB, 128, HD).transpose(1, 0, 2).reshape(128, -1)
                    ),
                    "woP": _bf16(
                        woT.reshape(GROUPS, 128, D).transpose(1, 0, 2).reshape(128, -1)
                    ),
                    "cosT": cosT,
                    "sinT": sinT,
                    "maskT": maskT,
                    "shiftPT": shiftPT,
                    "ident": identity,
                    "onescol": onescol,
                    "onesrow": onesrow,
                    "onesrowb": _bf16(onesrow),
                }
            )
    return in_maps


def kernel(x, Wq, Wk, Wv, Wo):
    global LAST_EXEC_NS, LAST_TRACE
    from concourse.bass_utils import run_bass_kernel_spmd

    if "nc" not in _CACHE:
        _CACHE["nc"] = _build_program()
    nc = _CACHE["nc"]

    x = np.asarray(x)
    in_maps = _per_core_inputs(
        x, np.asarray(Wq), np.asarray(Wk), np.asarray(Wv), np.asarray(Wo)
    )
    trace = bool(os.environ.get("KERNEL_PROFILE"))
    res = run_bass_kernel_spmd(
        nc, in_maps, core_ids=list(range(NCORES)), trace=trace
    )
    globals()["LAST_RESULT"] = res
    LAST_EXEC_NS = res.exec_time_ns
    LAST_TRACE = getattr(res, "profile_json", None)
    out = np.empty((NB, S, D), dtype=np.float32)
    for b in range(NB):
        acc = res.results[b * NKVH]["y"].astype(np.float32, copy=True)
        for g in range(1, NKVH):
            acc += res.results[b * NKVH + g]["y"]
        out[b] = acc
    return out


# revision 24
# speedup vs baseline: 1.0206x; 1.0206x over previous
"""Grouped-Query Attention (B=2, S=2048, D=2048, 16 Q heads / 4 KV heads,
hd=128, RoPE, causal) on 8 trn2 NeuronCores.

Sharding: mesh = 2 (batch) x 4 (KV-head groups).  Core c = b*4 + g gets
batch b and KV head g together with its 4 query heads (tensor parallel on
the head dim: q/k/v projection output dim and o-proj input dim).  Each core
produces a partial y[b] (o-proj over its 512 input dims); host sums the 4
partials per batch.

v3: all matmul operands bf16 (1 cycle/row + fast weight load; absmax rel
err ~4e-3 vs the 2e-2 gate).  Phase 1 serializes the contraction per
projection output (2 PSUM banks live) and lags each output's RoPE /
v-transpose epilogue one 16-matmul block behind, so the PE never waits on
ACT/DVE.  Phase 2 lags the softmax-denominator and attention@V matmuls of
head h-1 into head h's scores block (3 interleaved PE streams), and the
1/den -> broadcast -> normalize chain a further block, so the PE never
waits on the exp chain.  The reciprocal runs as ACT ln->exp(-x) on the
small early sq blocks (DVE is the scarce engine there) and as DVE
reciprocal on the big ones (ACT is scarce there).  DMAs are chunked and
priority-ordered so the first matmul issues ~12us in (8.6us of that is
fixed NEFF preamble).
"""

import os

import numpy as np

S = 2048
D = 2048
HD = 128
NQH = 16
NKVH = 4
GROUPS = NQH // NKVH  # 4 q heads per kv head
O = GROUPS * HD  # 512 per-core q/o slice
NB = 2
NCORES = 8
SCALE = 1.0 / float(np.sqrt(np.float32(HD)))
NEG = -1.0e30

SBLK = 512  # seq block for projections / sq block in attention
NKB = S // HD  # 16 128-blocks along seq
NSB = S // SBLK  # 4 512-blocks along seq
NDB = D // HD  # 16 d blocks

LAST_EXEC_NS = None
LAST_TRACE = None

_CACHE = {}


def _rope_tables():
    k = np.arange(0, HD, 2)[: HD // 2].astype(np.float32)
    inv_freq = (1.0 / 10000.0 ** (k / HD)).astype(np.float32)
    positions = np.arange(S, dtype=np.float32)
    ang = positions[:, None] * inv_freq[None, :]  # [S, 64]
    ang = np.concatenate([ang, ang], axis=-1)  # [S, 128]
    cosT = np.cos(ang).astype(np.float32).T  # [128, S]
    sinT = np.sin(ang).astype(np.float32).T
    return np.ascontiguousarray(cosT), np.ascontiguousarray(sinT)


def _mask_table():
    # maskT[i, j*512 + s] = 0 if (j*128 + i) <= s else NEG
    m = np.empty((HD, 4 * SBLK), dtype=np.float32)
    i = np.arange(HD)[:, None]
    s = np.arange(SBLK)[None, :]
    for j in range(4):
        m[:, j * SBLK : (j + 1) * SBLK] = np.where(j * HD + i <= s, 0.0, NEG)
    return m


def _shift_table():
    # rot = P @ q  with rot[i] = -q[i+64] (i<64), q[i-64] (i>=64); ship P.T
    P = np.zeros((HD, HD), dtype=np.float32)
    h = HD // 2
    P[np.arange(h), np.arange(h) + h] = -1.0
    P[np.arange(h) + h, np.arange(h)] = 1.0
    return np.ascontiguousarray(P.T)


def _build_program():
    import concourse.bass as bass
    import concourse.mybir as mybir
    from concourse.tile import TileContext

    f32 = mybir.dt.float32
    f32r = mybir.dt.float32r
    bf16 = mybir.dt.bfloat16
    EXP = mybir.ActivationFunctionType.Exp
    LN = mybir.ActivationFunctionType.Ln

    nc = bass.Bass()

    xP = nc.declare_dram_parameter("xP", [128, NSB * NDB * SBLK], bf16, isOutput=False)
    wqP = nc.declare_dram_parameter(
        "wqP", [128, GROUPS * NDB * HD], bf16, isOutput=False
    )
    wkP = nc.declare_dram_parameter("wkP", [128, NDB * HD], bf16, isOutput=False)
    wvP = nc.declare_dram_parameter("wvP", [128, NDB * HD], bf16, isOutput=False)
    woP = nc.declare_dram_parameter("woP", [128, GROUPS * D], bf16, isOutput=False)
    cosT = nc.declare_dram_parameter("cosT", [HD, S], bf16, isOutput=False)
    sinT = nc.declare_dram_parameter("sinT", [HD, S], bf16, isOutput=False)
    maskT = nc.declare_dram_parameter("maskT", [HD, 4 * SBLK], bf16, isOutput=False)
    shiftPT = nc.declare_dram_parameter("shiftPT", [HD, HD], bf16, isOutput=False)
    ident = nc.declare_dram_parameter("ident", [HD, HD], bf16, isOutput=False)
    onescol = nc.declare_dram_parameter("onescol", [HD, 1], bf16, isOutput=False)
    onesrow = nc.declare_dram_parameter("onesrow", [1, HD], f32r, isOutput=False)
    onesrowb = nc.declare_dram_parameter("onesrowb", [1, HD], bf16, isOutput=False)
    y = nc.declare_dram_parameter("y", [S, D], f32, isOutput=True)

    with TileContext(nc) as tc:
        with tc.tile_pool(name="persist", bufs=1) as pp:
            wq_sb = pp.tile([128, GROUPS * NDB * HD], bf16, name="wq_sb")
            wk_sb = pp.tile([128, NDB * HD], bf16, name="wk_sb")
            wv_sb = pp.tile([128, NDB * HD], bf16, name="wv_sb")
            wo_sb = pp.tile([128, GROUPS * D], bf16, name="wo_sb")
            cos_sb = pp.tile([128, S], bf16, name="cos_sb")
            sin_sb = pp.tile([128, S], bf16, name="sin_sb")
            mask_sb = pp.tile([128, 4 * SBLK], bf16, name="mask_sb")
            shift_sb = pp.tile([128, HD], bf16, name="shift_sb")
            id_sb = pp.tile([128, HD], bf16, name="id_sb")
            ones_sb = pp.tile([128, 1], bf16, name="ones_sb")
            oner_sb = pp.tile([1, HD], f32r, name="oner_sb")
            onerb_sb = pp.tile([1, HD], bf16, name="onerb_sb")
            q_sb = pp.tile([128, GROUPS * S], bf16, name="q_sb")  # per head [hd, S]
            k_sb = pp.tile([128, S], bf16, name="k_sb")
            v_sb = pp.tile([128, NKB * HD], bf16, name="v_sb")  # [s_blk][128s, hd]

            # ---- priority-ordered input DMAs.  First compute block is the
            # v projection of sb0, so wv + the first x chunks come first.
            nc.sync.dma_start(out=wv_sb[:], in_=wvP[:])

            with (
                tc.tile_pool(name="xts", bufs=2) as xpool,
                tc.tile_pool(name="p1acc", bufs=3, space="PSUM") as accp,
                tc.tile_pool(name="p1vacc", bufs=2, space="PSUM") as vaccp,
                tc.tile_pool(name="p1rot", bufs=2, space="PSUM") as rotp,
                tc.tile_pool(name="p1vt", bufs=1, space="PSUM") as vtp,
                tc.tile_pool(name="raws", bufs=3) as rawpool,
                tc.tile_pool(name="tmps", bufs=3) as tmppool,
            ):
                xsb0 = xpool.tile([128, NDB * SBLK], bf16, name="xt0", tag="xt")
                for c in range(8):
                    nc.sync.dma_start(
                        out=xsb0[:, c * 1024 : (c + 1) * 1024],
                        in_=xP[:, c * 1024 : (c + 1) * 1024],
                    )
                    if c == 1:
                        nc.sync.dma_start(
                            out=wq_sb[:, 0:2048], in_=wqP[:, 0:2048]
                        )  # q head 0
                    if c == 3:
                        nc.sync.dma_start(out=id_sb[:], in_=ident[:])
                        nc.sync.dma_start(out=cos_sb[:, 0:SBLK], in_=cosT[:, 0:SBLK])
                    if c == 5:
                        nc.sync.dma_start(out=sin_sb[:, 0:SBLK], in_=sinT[:, 0:SBLK])
                        nc.sync.dma_start(out=shift_sb[:], in_=shiftPT[:])
                for ob in range(1, GROUPS):
                    nc.sync.dma_start(
                        out=wq_sb[:, ob * 2048 : (ob + 1) * 2048],
                        in_=wqP[:, ob * 2048 : (ob + 1) * 2048],
                    )
                nc.sync.dma_start(out=wk_sb[:], in_=wkP[:])
                nc.sync.dma_start(out=ones_sb[:], in_=onescol[:])
                nc.sync.dma_start(out=oner_sb[:], in_=onesrow[:])
                nc.sync.dma_start(out=onerb_sb[:], in_=onesrowb[:])

                def rope(ps, raw_name, dst, seq_sl):
                    # dst = raw*cos + (P@raw)*sin, all bf16 except rot PSUM f32
                    raw = rawpool.tile([128, SBLK], bf16, name=raw_name, tag="raw")
                    nc.scalar.copy(raw[:], ps[:])
                    rot = rotp.tile([128, SBLK], f32, name="rot_" + raw_name, tag="rot")
                    nc.tensor.matmul(rot[:], shift_sb[:], raw[:], start=True, stop=True)
                    d = dst[0][:, dst[1]]
                    nc.vector.tensor_mul(d, raw[:], cos_sb[:, seq_sl])
                    t2 = tmppool.tile([128, SBLK], bf16, name="t2_" + raw_name, tag="t2")
                    nc.vector.tensor_mul(t2[:], rot[:], sin_sb[:, seq_sl])
                    nc.vector.tensor_add(d, d, t2[:])

                def vfinish(pv, sb):
                    # stage v to SBUF bf16, PE-transpose to [seq, hd] blocks
                    vst = rawpool.tile([128, SBLK], bf16, name=f"vst{sb}", tag="raw")
                    nc.scalar.copy(vst[:], pv[:])
                    vt = vtp.tile([128, SBLK], bf16, name=f"vt{sb}", tag="vt")
                    for sub in range(SBLK // HD):
                        nc.tensor.transpose(
                            vt[:, sub * HD : (sub + 1) * HD],
                            vst[:, sub * HD : (sub + 1) * HD],
                            id_sb[:],
                        )
                    nc.scalar.copy(v_sb[:, sb * SBLK : (sb + 1) * SBLK], vt[:])

                # block order per sb: v, q0..q3, k (sb0 fuses v+q0 so early
                # compute matches x-DMA delivery; sb3 puts v last so the final
                # phase-1 epilogue is the v transpose, which phase 2 does not
                # read until several blocks in).  Each block's epilogue (RoPE
                # or v transpose) is emitted after a LATER block's matmuls via
                # a FIFO so the PE never waits on ACT/DVE.
                taskq = []

                def flush_one():
                    if taskq:
                        fn, args = taskq.pop(0)
                        fn(*args)

                for sb in range(NSB):
                    sl = slice(sb * SBLK, (sb + 1) * SBLK)
                    if sb > 0:
                        xsb = xpool.tile([128, NDB * SBLK], bf16, name=f"xt{sb}", tag="xt")
                        for c in range(4):
                            nc.sync.dma_start(
                                out=xsb[:, c * 2048 : (c + 1) * 2048],
                                in_=xP[
                                    :, sb * 8192 + c * 2048 : sb * 8192 + (c + 1) * 2048
                                ],
                            )
                        nc.sync.dma_start(out=cos_sb[:, sl], in_=cosT[:, sl])
                        nc.sync.dma_start(out=sin_sb[:, sl], in_=sinT[:, sl])
                        if sb == 1:
                            nc.sync.dma_start(out=mask_sb[:], in_=maskT[:])
                        if sb == 2:
                            nc.sync.dma_start(out=wo_sb[:], in_=woP[:])
                    else:
                        xsb = xsb0

                    def vblock(sb=sb, xsb=xsb):
                        pv = vaccp.tile([128, SBLK], f32, name=f"vacc{sb}", tag="vacc")
                        for db in range(NDB):
                            nc.tensor.matmul(
                                pv[:],
                                wv_sb[:, db * HD : (db + 1) * HD],
                                xsb[:, db * SBLK : (db + 1) * SBLK],
                                start=(db == 0),
                                stop=(db == NDB - 1),
                            )
                        return pv

                    oi_start = 0
                    if sb == 0:
                        # fused v + q0 contraction over the incoming x chunks
                        pv = vaccp.tile([128, SBLK], f32, name="vacc0", tag="vacc")
                        ps0 = accp.tile([128, SBLK], f32, name="acc0_0", tag="acc")
                        for db in range(NDB):
                            nc.tensor.matmul(
                                pv[:],
                                wv_sb[:, db * HD : (db + 1) * HD],
                                xsb[:, db * SBLK : (db + 1) * SBLK],
                                start=(db == 0),
                                stop=(db == NDB - 1),
                            )
                            nc.tensor.matmul(
                                ps0[:],
                                wq_sb[:, db * HD : (db + 1) * HD],
                                xsb[:, db * SBLK : (db + 1) * SBLK],
                                start=(db == 0),
                                stop=(db == NDB - 1),
                            )
                        taskq.append((vfinish, (pv, sb)))
                        taskq.append((rope, (ps0, "r0_0", (q_sb, slice(0, SBLK)), sl)))
                        oi_start = 1
                    elif sb < 3:
                        pv = vblock()
                        flush_one()
                        taskq.append((vfinish, (pv, sb)))

                    # q heads then k: serialized 16-step contraction per output
                    for oi in range(oi_start, 5):
                        ps = accp.tile([128, SBLK], f32, name=f"acc{sb}_{oi}", tag="acc")
                        for db in range(NDB):
                            if oi < 4:
                                w = wq_sb[
                                    :, oi * 2048 + db * HD : oi * 2048 + (db + 1) * HD
                                ]
                            else:
                                w = wk_sb[:, db * HD : (db + 1) * HD]
                            nc.tensor.matmul(
                                ps[:],
                                w,
                                xsb[:, db * SBLK : (db + 1) * SBLK],
                                start=(db == 0),
                                stop=(db == NDB - 1),
                            )
                        flush_one()
                        if sb == 3 and oi < 2:
                            flush_one()
                        dst = (
                            (q_sb, slice(oi * S + sb * SBLK, oi * S + (sb + 1) * SBLK))
                            if oi < 4
                            else (k_sb, sl)
                        )
                        taskq.append((rope, (ps, f"r{sb}_{oi}", dst, sl)))
                    if sb == 3:
                        pv = vblock()
                        flush_one()
                        taskq.append((vfinish, (pv, sb)))
                while taskq:
                    flush_one()

            # ---------------- Phase 2: attention + o-proj
            with (
                tc.tile_pool(name="p2sc", bufs=3, space="PSUM") as scp,
                tc.tile_pool(name="p2av", bufs=2, space="PSUM") as avp,
                tc.tile_pool(name="p2den", bufs=2, space="PSUM") as denp,
                tc.tile_pool(name="p2bc", bufs=1, space="PSUM") as bcp,
                tc.tile_pool(name="exps", bufs=36) as epool,
                tc.tile_pool(name="denrs", bufs=4) as drpool,
                tc.tile_pool(name="bcsb", bufs=2) as bcsbpool,
                tc.tile_pool(name="aos", bufs=2) as aopool,
                tc.tile_pool(name="ysb", bufs=3) as ypool_sb,
            ):
                ao_tiles = {}

                def emit_recip(sq, h, den, use_act=None):
                    # 1/den: ACT ln->exp(-x) on small sq (DVE-scarce blocks),
                    # DVE reciprocal on big sq (ACT-scarce blocks)
                    if use_act is None:
                        use_act = True
                    if use_act:
                        lden = bcsbpool.tile(
                            [1, SBLK], f32, name=f"ld{sq}_{h}", tag="ld"
                        )
                        nc.scalar.activation(lden[:], den[:], LN)
                        denr = drpool.tile(
                            [1, SBLK], bf16, name=f"denr{sq}_{h}", tag="denr"
                        )
                        nc.scalar.activation(denr[:], lden[:], EXP, scale=-1.0)
                        return (denr, onerb_sb)
                    denr = drpool.tile([1, SBLK], f32r, name=f"denr{sq}_{h}", tag="denr")
                    with nc.allow_low_precision(reason="f32r softmax denom"):
                        nc.vector.reciprocal(denr[:], den[:])
                    return (denr, oner_sb)

                def finish_norm(rec):
                    # bc = broadcast(1/den) via ones-column matmul; ao = av * bc
                    psq, ph, av, (denr, oner) = rec
                    bc = bcp.tile([128, SBLK], f32, name=f"bc{psq}_{ph}", tag="bc")
                    nc.tensor.matmul(bc[:], oner[:], denr[:], start=True, stop=True)
                    bcs = bcsbpool.tile([128, SBLK], f32, name=f"bcs{psq}_{ph}", tag="bcs")
                    nc.scalar.copy(bcs[:], bc[:])
                    nc.vector.tensor_mul(
                        ao_tiles[psq][:, ph * SBLK : (ph + 1) * SBLK], av[:], bcs[:]
                    )

                def emit_oproj(sq):
                    ao = ao_tiles[sq]
                    for sub in range(SBLK // HD):
                        for dc in range(D // SBLK):
                            yt = scp.tile(
                                [128, SBLK], f32, name=f"y{sq}_{sub}_{dc}", tag="sc"
                            )
                            for ob in range(GROUPS):
                                nc.tensor.matmul(
                                    yt[:],
                                    ao[
                                        :, ob * SBLK + sub * HD : ob * SBLK + (sub + 1) * HD
                                    ],
                                    wo_sb[:, ob * D + dc * SBLK : ob * D + (dc + 1) * SBLK],
                                    start=(ob == 0),
                                    stop=(ob == GROUPS - 1),
                                )
                            ysb = ypool_sb.tile(
                                [128, SBLK], f32, name=f"ysb{sq}_{sub}_{dc}", tag="ysb"
                            )
                            if (sub + dc) % 2 == 0:
                                nc.scalar.copy(ysb[:], yt[:])
                            else:
                                nc.vector.tensor_copy(ysb[:], yt[:])
                            nc.sync.dma_start(
                                out=y[
                                    sq * SBLK + sub * HD : sq * SBLK + (sub + 1) * HD,
                                    dc * SBLK : (dc + 1) * SBLK,
                                ],
                                in_=ysb[:],
                            )

                lagA = None  # (sq, h, es): den+av matmuls run in current block
                lagN = None  # (sq, h, av, rec): bc/bcs/normalize run in current block
                for sq in range(NSB):
                    nsk = 4 * sq + 4
                    ao_tiles[sq] = aopool.tile(
                        [128, GROUPS * SBLK], bf16, name=f"ao{sq}", tag="ao"
                    )
                    for h in range(GROUPS):
                        es = []
                        if lagA is not None:
                            asq, ah, aes = lagA
                            psk = len(aes)
                            den = denp.tile([1, SBLK], f32, name=f"den{asq}_{ah}", tag="den")
                            av = avp.tile([128, SBLK], f32, name=f"av{asq}_{ah}", tag="av")
                        else:
                            psk = 0
                        qcols = q_sb[:, h * S + sq * SBLK : h * S + (sq + 1) * SBLK]
                        # columns below j*128 of diagonal tile j are fully
                        # masked (exp -> 0): trim scores/mask/exp/den/av to the
                        # valid column subrange.  es[i] = (tile, col_offset).
                        def co_of(sq_, i_):
                            return max(0, (i_ - 4 * sq_) * HD)

                        for i in range(max(nsk, psk)):
                            if i < nsk:
                                co = co_of(sq, i)
                                sc = scp.tile(
                                    [128, SBLK], f32, name=f"sc{sq}_{h}_{i}", tag="sc"
                                )
                                nc.tensor.matmul(
                                    sc[:, co:SBLK],
                                    k_sb[:, i * HD : (i + 1) * HD],
                                    qcols[:, co:SBLK],
                                    start=True,
                                    stop=True,
                                )
                                if i >= 4 * sq:
                                    j = i - 4 * sq
                                    nc.vector.tensor_add(
                                        sc[:, co:SBLK],
                                        sc[:, co:SBLK],
                                        mask_sb[:, j * SBLK + co : (j + 1) * SBLK],
                                    )
                            if i < psk:
                                pco = co_of(asq, i)
                                nc.tensor.matmul(
                                    den[:, pco:SBLK],
                                    ones_sb[:],
                                    aes[i][0][:, pco:SBLK],
                                    start=(i == 0),
                                    stop=(i == psk - 1),
                                )
                                nc.tensor.matmul(
                                    av[:, pco:SBLK],
                                    v_sb[:, i * HD : (i + 1) * HD],
                                    aes[i][0][:, pco:SBLK],
                                    start=(i == 0),
                                    stop=(i == psk - 1),
                                )
                            if i < nsk:
                                e = epool.tile(
                                    [128, SBLK], bf16, name=f"e{sq}_{h}_{i}", tag="e"
                                )
                                nc.scalar.activation(
                                    e[:, co:SBLK], sc[:, co:SBLK], EXP, scale=SCALE
                                )
                                es.append((e, co))
                        if lagA is not None:
                            rec = emit_recip(asq, ah, den)
                            if lagN is not None:
                                finish_norm(lagN)
                            lagN = (asq, ah, av, rec)
                        lagA = (sq, h, es)
                        if h == 1 and sq > 0:
                            emit_oproj(sq - 1)
                # drain: den+av for (3,3), last normalizations, final o-proj
                asq, ah, aes = lagA
                psk = len(aes)
                den = denp.tile([1, SBLK], f32, name="den3_3", tag="den")
                av = avp.tile([128, SBLK], f32, name="av3_3", tag="av")
                for i in range(psk):
                    pco = max(0, (i - 4 * asq) * HD)
                    nc.tensor.matmul(
                        den[:, pco:SBLK],
                        ones_sb[:],
                        aes[i][0][:, pco:SBLK],
                        start=(i == 0),
                        stop=(i == psk - 1),
                    )
                rec = emit_recip(asq, ah, den, use_act=True)
                for i in range(psk):
                    pco = max(0, (i - 4 * asq) * HD)
                    nc.tensor.matmul(
                        av[:, pco:SBLK],
                        v_sb[:, i * HD : (i + 1) * HD],
                        aes[i][0][:, pco:SBLK],
                        start=(i == 0),
                        stop=(i == psk - 1),
                    )
                finish_norm(lagN)
                finish_norm((asq, ah, av, rec))
                emit_oproj(NSB - 1)

    _split_matmul_waits(nc, mybir)
    return nc


def _split_matmul_waits(nc, mybir):
    """TRN2 instructions can carry only one HW sync-wait command; Tile
    sometimes attaches several.  Move the extras onto nofuse nops on the
    same engine inserted just before the instruction."""
    for f in nc.m.functions:
        for bb in f.blocks:
            insts = bb.instructions
            fixes = []
            for idx, inst in enumerate(insts):
                si = inst.sync_info
                if si is None or len(si.on_wait) <= 1:
                    continue
                fixes.append((idx, inst, list(si.on_wait), list(si.on_update)))
            for idx, inst, waits, updates in reversed(fixes):
                inst.sync_info = mybir.SyncInfo(on_wait=[waits[-1]], on_update=updates)
                for w in reversed(waits[:-1]):
                    nop = mybir.InstNoOp(
                        name=nc.get_next_instruction_name(), ins=[], outs=[]
                    )
                    nop.engine = inst.engine
                    nop.bass_nofuse = True
                    nop.sync_info = mybir.SyncInfo(on_wait=[w], on_update=[])
                    insts.insert(idx, nop)


def _bf16(a):
    import ml_dtypes

    return np.ascontiguousarray(a).astype(ml_dtypes.bfloat16)


def _per_core_inputs(x, Wq, Wk, Wv, Wo):
    cosT, sinT = _rope_tables()
    maskT = _mask_table()
    shiftPT = _shift_table()
    identity = np.eye(HD, dtype=np.float32)
    onescol = np.ones((HD, 1), dtype=np.float32)
    onesrow = np.ones((1, HD), dtype=np.float32)
    cosT = _bf16(cosT)
    sinT = _bf16(sinT)
    maskT = _bf16(maskT)
    shiftPT = _bf16(shiftPT)
    identity = _bf16(identity)
    onescol = _bf16(onescol)
    in_maps = []
    for b in range(NB):
        xT = x[b].T.astype(np.float32, copy=False)  # [D, S]
        xPb = _bf16(
            xT.reshape(NDB, 128, NSB, SBLK).transpose(1, 2, 0, 3).reshape(128, -1)
        )
        for g in range(NKVH):
            wqT = Wq[g * O : (g + 1) * O, :].T  # [D, O]
            wkT = Wk[g * HD : (g + 1) * HD, :].T  # [D, HD]
            wvT = Wv[g * HD : (g + 1) * HD, :].T
            woT = Wo[:, g * O : (g + 1) * O].T  # [O, D]
            in_maps.append(
                {
                    "xP": xPb,
                    "wqP": _bf16(
                        wqT.reshape(NDB, 128, GROUPS, HD)
                        .transpose(1, 2, 0, 3)
                        .reshape(128, -1)
                    ),
                    "wkP": _bf16(
                        wkT.reshape(NDB, 128, HD).transpose(1, 0, 2).reshape(128, -1)
                    ),
                    "wvP": _bf16(
                        wvT.reshape(NDB, 128, HD).transpose(1, 0, 2).reshape(128, -1)
                    ),
                    "woP": _bf16(
                        woT.reshape(GROUPS, 128, D).transpose(1, 0, 2).reshape(128, -1)
                    ),
                    "cosT": cosT,
                    "sinT": sinT,
                    "maskT": maskT,
                    "shiftPT": shiftPT,
                    "ident": identity,
                    "onescol": onescol,
                    "onesrow": onesrow,
                    "onesrowb": _bf16(onesrow),
                }
            )
    return in_maps


def kernel(x, Wq, Wk, Wv, Wo):
    global LAST_EXEC_NS, LAST_TRACE
    from concourse.bass_utils import run_bass_kernel_spmd

    if "nc" not in _CACHE:
        _CACHE["nc"] = _build_program()
    nc = _CACHE["nc"]

    x = np.asarray(x)
    in_maps = _per_core_inputs(
        x, np.asarray(Wq), np.asarray(Wk), np.asarray(Wv), np.asarray(Wo)
    )
    trace = bool(os.environ.get("KERNEL_PROFILE"))
    res = run_bass_kernel_spmd(
        nc, in_maps, core_ids=list(range(NCORES)), trace=trace
    )
    globals()["LAST_RESULT"] = res
    LAST_EXEC_NS = res.exec_time_ns
    LAST_TRACE = getattr(res, "profile_json", None)
    out = np.empty((NB, S, D), dtype=np.float32)
    for b in range(NB):
        acc = res.results[b * NKVH]["y"].astype(np.float32, copy=True)
        for g in range(1, NKVH):
            acc += res.results[b * NKVH + g]["y"]
        out[b] = acc
    return out
